# revision 1
# baseline (speedup 1.0000x reference)
"""Binary (sign-sign) linear layer on 8 TRN2 NeuronCores.

out = sign(x) @ sign(w),  x: [8192, 4096] f32, w: [4096, 4096] f32.

Strategy:
  - Data-parallel shard of x rows (M) across 8 cores; w replicated.
  - Host prep: cast inputs to bf16 (sign-preserving; randn values are far
    from the bf16 subnormal range so sign() is unchanged) and pre-block the
    layouts so every DMA is a contiguous >=1KB-per-partition transfer.
  - On device: binarize to fp8 (e4m3 represents {-1, 0, +1} exactly) with
    ACT Sign / DVE clamp ops, then run the matmul with DoubleRow perf mode
    (2 fp8 MACs/cell/cycle, K=256 contraction per matmul).
  - Accumulation is fp32 in PSUM; all products are +-1/0 so the result is
    exact (integers |v| <= 4096).
"""

import numpy as np
import ml_dtypes

import concourse.bass as bass
import concourse.mybir as mybir
import concourse.tile as tile
from concourse import bacc
from concourse.bass_utils import run_bass_kernel_spmd

P = 128
N_BLK = 512  # PSUM bank free-dim width (fp32)
FP8 = mybir.dt.float8e4
BF16 = mybir.dt.bfloat16
F32 = mybir.dt.float32

N_CORES = 8


def build_nc(m_shard: int, K: int, N: int):
    """Build the single-core Bass program (same NEFF runs SPMD on all cores).

    DRAM inputs (per core):
      xt : [P, KO, m_shard] bf16, xt[p, ko, m] = x[m0 + m, ko*P + p]
      w  : [NB, P, KO, N_BLK] bf16, w[nb, p, ko, n] = w_full[ko*P + p, nb*N_BLK + n]
    DRAM output:
      out: [m_shard, N] f32
    """
    KO = K // P          # number of 128-row k subtiles
    PAIRS = KO // 2      # DoubleRow pairs
    NB = N // N_BLK      # n blocks
    MS = m_shard // P    # m subtiles
    XG = min(4, KO)      # k-subtiles per x staging DMA
    WG = min(8, KO)      # k-subtiles per w staging DMA

    nc = bacc.Bacc("TRN2", target_bir_lowering=False, debug=False)
    xt_d = nc.dram_tensor("xt", [P, KO, m_shard], BF16, kind="ExternalInput").ap()
    w_d = nc.dram_tensor("w", [NB, P, KO, N_BLK], BF16, kind="ExternalInput").ap()
    out_d = nc.dram_tensor("out", [m_shard, N], F32, kind="ExternalOutput").ap()

    with tile.TileContext(nc) as tc:
        with (
            tc.tile_pool(name="xstage", bufs=2) as xstage_pool,
            tc.tile_pool(name="xbt", bufs=1) as xbt_pool,
            tc.tile_pool(name="wstage", bufs=3) as wstage_pool,
            tc.tile_pool(name="wb", bufs=2) as wb_pool,
            tc.tile_pool(name="tmp", bufs=2) as tmp_pool,
            tc.tile_pool(name="outp", bufs=4) as out_pool,
            tc.tile_pool(name="psum", bufs=4, space="PSUM") as psum_pool,
        ):
            # --- binarize x into a persistent fp8 [P, KO, m_shard] tile ---
            xbt = xbt_pool.tile([P, KO, m_shard], FP8)
            for g in range(KO // XG):
                xs = xstage_pool.tile([P, XG, m_shard], BF16, name="xs")
                nc.sync.dma_start(xs[:], xt_d[:, g * XG : (g + 1) * XG, :])
                for h in range(XG // 2):
                    j = g * (XG // 2) + h  # global pair index
                    src = xs[:, 2 * h : 2 * h + 2, :]
                    dst = xbt[:, g * XG + 2 * h : g * XG + 2 * h + 2, :]
                    if j % 3 == 2:
                        # DVE path: sign(v) == min(max(v * 2^126, -1), 1)
                        # (exact for normals incl. 0; randn never subnormal)
                        t = tmp_pool.tile([P, 2, m_shard], BF16, name="xtmp")
                        nc.vector.tensor_scalar(
                            t[:], src, 2.0**126, -1.0,
                            mybir.AluOpType.mult, mybir.AluOpType.max,
                        )
                        nc.vector.tensor_scalar(
                            dst, t[:], 1.0, None, mybir.AluOpType.min
                        )
                    else:
                        nc.scalar.sign(dst, src)

            # --- loop over n blocks: binarize w strip, matmul all m tiles ---
            for nb in range(NB):
                wb = wb_pool.tile([P, KO, N_BLK], FP8, name="wb")
                for g in range(KO // WG):
                    ws = wstage_pool.tile([P, WG, N_BLK], BF16, name="ws")
                    nc.sync.dma_start(ws[:], w_d[nb, :, g * WG : (g + 1) * WG, :])
                    nc.scalar.sign(wb[:, g * WG : (g + 1) * WG, :], ws[:])
                for ms in range(MS):
                    ps = psum_pool.tile([P, N_BLK], F32, name="ps")
                    for j in range(PAIRS):
                        nc.tensor.matmul(
                            ps[:],
                            xbt[:, 2 * j : 2 * j + 2, ms * P : (ms + 1) * P],
                            wb[:, 2 * j : 2 * j + 2, :],
                            start=(j == 0),
                            stop=(j == PAIRS - 1),
                            perf_mode=mybir.MatmulPerfMode.DoubleRow,
                        )
                    ot = out_pool.tile([P, N_BLK], F32, name="ot")
                    nc.vector.tensor_copy(out=ot[:], in_=ps[:])
                    nc.sync.dma_start(
                        out_d[ms * P : (ms + 1) * P, nb * N_BLK : (nb + 1) * N_BLK],
                        ot[:],
                    )
    nc.compile()
    return nc


def host_prep(x: np.ndarray, weight: np.ndarray, n_cores: int = N_CORES):
    """Cast to bf16 and pre-block layouts; returns per-core input maps."""
    M, K = x.shape
    _, N = weight.shape
    m_shard = M // n_cores
    KO = K // P
    NB = N // N_BLK

    xb = x.astype(ml_dtypes.bfloat16)
    wb = weight.astype(ml_dtypes.bfloat16)

    # xt[p, ko, m_full] = x[m_full, ko*P + p]
    xt = np.ascontiguousarray(xb.T.reshape(KO, P, M).transpose(1, 0, 2))
    # w_blk[nb, p, ko, n] = w[ko*P + p, nb*N_BLK + n]
    w_blk = np.ascontiguousarray(
        wb.reshape(KO, P, NB, N_BLK).transpose(2, 1, 0, 3)
    )

    in_maps = [
        {
            "xt": np.ascontiguousarray(xt[:, :, c * m_shard : (c + 1) * m_shard]),
            "w": w_blk,
        }
        for c in range(n_cores)
    ]
    return in_maps, m_shard


_NC_CACHE: dict = {}


def get_nc(m_shard: int, K: int, N: int):
    key = (m_shard, K, N)
    if key not in _NC_CACHE:
        _NC_CACHE[key] = build_nc(m_shard, K, N)
    return _NC_CACHE[key]


def run(x: np.ndarray, weight: np.ndarray, **spmd_kwargs):
    """Shard, run on 8 cores, gather. Returns (output, BassKernelResults)."""
    in_maps, m_shard = host_prep(x, weight)
    nc = get_nc(m_shard, x.shape[1], weight.shape[1])
    res = run_bass_kernel_spmd(
        nc, in_maps, core_ids=list(range(N_CORES)), **spmd_kwargs
    )
    out = np.concatenate([r["out"] for r in res.results], axis=0)
    return out, res


def kernel(x: np.ndarray, weight: np.ndarray) -> np.ndarray:
    out, _ = run(x, weight)
    return out


# revision 2
# speedup vs baseline: 1.0201x; 1.0201x over previous
"""Binary (sign-sign) linear layer on 8 TRN2 NeuronCores.

out = sign(x) @ sign(w),  x: [8192, 4096] f32, w: [4096, 4096] f32.

Strategy:
  - Data-parallel shard of x rows (M) across 8 cores; w replicated.
  - Host prep: cast inputs to bf16 (sign-preserving; randn values are far
    from the bf16 subnormal range so sign() is unchanged) and pre-block the
    layouts so every DMA is a contiguous multi-KB-per-partition transfer.
  - On device: binarize to fp8 (e4m3 represents {-1, 0, +1} exactly) with
    ACT Sign / DVE clamp ops, then run the matmul with DoubleRow perf mode
    (2 fp8 MACs/cell/cycle, K=256 contraction per matmul).
  - Accumulation is fp32 in PSUM; all products are +-1/0 so the result is
    exact (integers |v| <= 4096).

Startup is the critical path (binarize supply racing the PE), so the
prologue is hand-scheduled: x-sign work is split across ACT and DVE in
deadline order, the first two n-blocks run k-pair-outer (so the PE only
needs pair j by MM 8*j instead of all pairs by MM 16), and a burst of
dummy matmuls warms the PE HAM clock gate during the prologue.
"""

import numpy as np
import ml_dtypes

import concourse.bass as bass
import concourse.mybir as mybir
import concourse.tile as tile
from concourse import bacc
from concourse.bass_utils import run_bass_kernel_spmd

P = 128
N_BLK = 512  # PSUM bank free-dim width (fp32)
FP8 = mybir.dt.float8e4
BF16 = mybir.dt.bfloat16
F32 = mybir.dt.float32

N_CORES = 8
WARMUP_MMS = 80


def build_nc(m_shard: int, K: int, N: int):
    """Build the single-core Bass program (same NEFF runs SPMD on all cores).

    DRAM inputs (per core):
      xt : [P, KO, m_shard] bf16, xt[p, ko, m] = x[m0 + m, ko*P + p]
      w  : [NB, P, KO, N_BLK] bf16, w[nb, p, ko, n] = w_full[ko*P + p, nb*N_BLK + n]
    DRAM output:
      out: [m_shard, N] f32
    """
    KO = K // P          # number of 128-row k subtiles
    PAIRS = KO // 2      # DoubleRow pairs
    NB = N // N_BLK      # n blocks
    MS = m_shard // P    # m subtiles
    XG = min(4, KO)      # k-subtiles per x staging DMA (2 pairs)
    WG = min(8, KO)      # k-subtiles per w staging DMA
    XGRP = KO // XG      # x staging groups
    WGRP = KO // WG      # w staging groups per n block
    # the hand-scheduled prologue below assumes the full-size shape
    full = (KO == 32 and NB == 8 and MS == 8)

    nc = bacc.Bacc("TRN2", target_bir_lowering=False, debug=False)
    xt_d = nc.dram_tensor("xt", [P, KO, m_shard], BF16, kind="ExternalInput").ap()
    w_d = nc.dram_tensor("w", [NB, P, KO, N_BLK], BF16, kind="ExternalInput").ap()
    out_d = nc.dram_tensor("out", [m_shard, N], F32, kind="ExternalOutput").ap()

    with tile.TileContext(nc) as tc:
        with (
            tc.tile_pool(name="xstage", bufs=5) as xstage_pool,
            tc.tile_pool(name="xbt", bufs=1) as xbt_pool,
            tc.tile_pool(name="wstage", bufs=3) as wstage_pool,
            tc.tile_pool(name="wb", bufs=3) as wb_pool,
            tc.tile_pool(name="tmp", bufs=2) as tmp_pool,
            tc.tile_pool(name="wtmp", bufs=2) as wtmp_pool,
            tc.tile_pool(name="outp", bufs=4) as out_pool,
            tc.tile_pool(name="const", bufs=1) as const_pool,
            tc.tile_pool(name="psum", bufs=8, space="PSUM") as psum_pool,
        ):
            xbt = xbt_pool.tile([P, KO, m_shard], FP8)
            xs_tiles: list = [None] * XGRP
            wb_tiles: dict = {}
            ws_tiles: dict = {}

            def x_dma(g):
                xs = xstage_pool.tile([P, XG, m_shard], BF16, name="xs")
                nc.sync.dma_start(xs[:], xt_d[:, g * XG : (g + 1) * XG, :])
                xs_tiles[g] = xs

            def w_dma(nb, g):
                if nb not in wb_tiles:
                    wb_tiles[nb] = wb_pool.tile([P, KO, N_BLK], FP8, name="wb")
                ws = wstage_pool.tile([P, WG, N_BLK], BF16, name="ws")
                nc.sync.dma_start(ws[:], w_d[nb, :, g * WG : (g + 1) * WG, :])
                ws_tiles[(nb, g)] = ws

            def x_sign_act(p):
                g, h = p // (XG // 2), p % (XG // 2)
                nc.scalar.sign(
                    xbt[:, 2 * p : 2 * p + 2, :],
                    xs_tiles[g][:, 2 * h : 2 * h + 2, :],
                )

            def x_sign_dve(p):
                # sign(v) == min(max(v * 2^126, -1), 1) for non-subnormal v
                g, h = p // (XG // 2), p % (XG // 2)
                src = xs_tiles[g][:, 2 * h : 2 * h + 2, :]
                t = tmp_pool.tile([P, 2, m_shard], BF16, name="xtmp")
                nc.vector.tensor_scalar(
                    t[:], src, 2.0**126, -1.0,
                    mybir.AluOpType.mult, mybir.AluOpType.max,
                )
                nc.vector.tensor_scalar(
                    xbt[:, 2 * p : 2 * p + 2, :], t[:], 1.0, None,
                    mybir.AluOpType.min,
                )

            def w_sign_act(nb, g):
                nc.scalar.sign(
                    wb_tiles[nb][:, g * WG : (g + 1) * WG, :],
                    ws_tiles.pop((nb, g))[:],
                )

            def w_sign_dve(nb, g):
                t = wtmp_pool.tile([P, WG, N_BLK], BF16, name="wtmp")
                nc.vector.tensor_scalar(
                    t[:], ws_tiles.pop((nb, g))[:], 2.0**126, -1.0,
                    mybir.AluOpType.mult, mybir.AluOpType.max,
                )
                nc.vector.tensor_scalar(
                    wb_tiles[nb][:, g * WG : (g + 1) * WG, :], t[:], 1.0, None,
                    mybir.AluOpType.min,
                )

            def w_prep(nb):
                for g in range(WGRP):
                    w_dma(nb, g)
                for g in range(WGRP - 1):
                    w_sign_act(nb, g)
                w_sign_dve(nb, WGRP - 1)

            def mm(ps, nb, j, ms, start, stop):
                nc.tensor.matmul(
                    ps[:],
                    xbt[:, 2 * j : 2 * j + 2, ms * P : (ms + 1) * P],
                    wb_tiles[nb][:, 2 * j : 2 * j + 2, :],
                    start=start,
                    stop=stop,
                    perf_mode=mybir.MatmulPerfMode.DoubleRow,
                )

            def copyback_store(ps, nb, ms):
                ot = out_pool.tile([P, N_BLK], F32, name="ot")
                nc.vector.tensor_copy(out=ot[:], in_=ps[:])
                nc.sync.dma_start(
                    out_d[ms * P : (ms + 1) * P, nb * N_BLK : (nb + 1) * N_BLK],
                    ot[:],
                )

            def nb_jouter(nb):
                ps = [psum_pool.tile([P, N_BLK], F32, name="ps") for _ in range(MS)]
                for j in range(PAIRS):
                    for ms in range(MS):
                        mm(ps[ms], nb, j, ms, j == 0, j == PAIRS - 1)
                for ms in range(MS):
                    copyback_store(ps[ms], nb, ms)

            def nb_msinner(nb):
                for ms in range(MS):
                    ps = psum_pool.tile([P, N_BLK], F32, name="ps")
                    for j in range(PAIRS):
                        mm(ps, nb, j, ms, j == 0, j == PAIRS - 1)
                    copyback_store(ps, nb, ms)

            # ---------------- prologue ----------------
            if full:
                # PE warmup: dummy matmuls during the prologue keep the HAM
                # activity window busy so real matmuls start at 2.4 GHz.
                dummy = const_pool.tile([P, P], BF16)
                nc.gpsimd.memset(dummy[:], 0.0)
                dps = [psum_pool.tile([P, N_BLK], F32, name="ps") for _ in range(2)]
                for i in range(WARMUP_MMS):
                    nc.tensor.matmul(
                        dps[i % 2][:, :64], dummy[:], dummy[:, :64],
                        start=True, stop=True,
                    )

                # sync DMA queue: interleave x and w loads in deadline order
                w_dma(0, 0)
                x_dma(0)
                w_dma(0, 1)
                x_dma(1)
                w_dma(0, 2)
                x_dma(2)
                w_dma(0, 3)
                x_dma(3)
                w_dma(1, 0)
                x_dma(4)
                w_dma(1, 1)
                x_dma(5)
                w_dma(1, 2)
                x_dma(6)
                w_dma(1, 3)
                x_dma(7)
                # ACT queue (sign = 1 inst): wb0 groups + late x pairs
                # DVE queue (clamp = 2 inst): early x pairs + g3 groups
                x_sign_dve(0)
                w_sign_act(0, 0)
                x_sign_act(6)
                x_sign_act(7)
                x_sign_dve(1)
                x_sign_dve(2)
                w_sign_act(0, 1)
                x_sign_act(8)
                x_sign_act(9)
                x_sign_dve(3)
                x_sign_dve(4)
                w_sign_act(0, 2)
                x_sign_dve(5)
                w_sign_dve(0, 3)
                x_sign_act(10)
                x_sign_act(11)
                x_sign_act(12)
                x_sign_act(13)
                x_sign_act(14)
                w_sign_dve(1, 3)
                w_sign_act(1, 0)
                x_sign_dve(15)
                w_sign_act(1, 1)
                w_sign_act(1, 2)

                # n blocks: first two k-pair-outer (progressive pair needs),
                # rest m-subtile-inner; each block prefetches w two ahead.
                w_prep(2)
                nb_jouter(0)
                w_prep(3)
                nb_jouter(1)
                for nb in range(2, NB):
                    if nb + 2 < NB:
                        w_prep(nb + 2)
                    nb_msinner(nb)
            else:
                # generic small-shape path (simulator testing)
                for g in range(XGRP):
                    x_dma(g)
                for p in range(PAIRS):
                    if p % 3 == 2:
                        x_sign_dve(p)
                    else:
                        x_sign_act(p)
                for nb in range(NB):
                    w_prep(nb)
                    nb_msinner(nb)
    nc.compile()
    return nc


def host_prep(x: np.ndarray, weight: np.ndarray, n_cores: int = N_CORES):
    """Cast to bf16 and pre-block layouts; returns per-core input maps."""
    M, K = x.shape
    _, N = weight.shape
    m_shard = M // n_cores
    KO = K // P
    NB = N // N_BLK

    xb = x.astype(ml_dtypes.bfloat16)
    wb = weight.astype(ml_dtypes.bfloat16)

    # xt[p, ko, m_full] = x[m_full, ko*P + p]
    xt = np.ascontiguousarray(xb.T.reshape(KO, P, M).transpose(1, 0, 2))
    # w_blk[nb, p, ko, n] = w[ko*P + p, nb*N_BLK + n]
    w_blk = np.ascontiguousarray(
        wb.reshape(KO, P, NB, N_BLK).transpose(2, 1, 0, 3)
    )

    in_maps = [
        {
            "xt": np.ascontiguousarray(xt[:, :, c * m_shard : (c + 1) * m_shard]),
            "w": w_blk,
        }
        for c in range(n_cores)
    ]
    return in_maps, m_shard


_NC_CACHE: dict = {}


def get_nc(m_shard: int, K: int, N: int):
    key = (m_shard, K, N)
    if key not in _NC_CACHE:
        _NC_CACHE[key] = build_nc(m_shard, K, N)
    return _NC_CACHE[key]


def run(x: np.ndarray, weight: np.ndarray, **spmd_kwargs):
    """Shard, run on 8 cores, gather. Returns (output, BassKernelResults)."""
    in_maps, m_shard = host_prep(x, weight)
    nc = get_nc(m_shard, x.shape[1], weight.shape[1])
    res = run_bass_kernel_spmd(
        nc, in_maps, core_ids=list(range(N_CORES)), **spmd_kwargs
    )
    out = np.concatenate([r["out"] for r in res.results], axis=0)
    return out, res


def kernel(x: np.ndarray, weight: np.ndarray) -> np.ndarray:
    out, _ = run(x, weight)
    return out
